# revision 1
# baseline (speedup 1.0000x reference)
"""Trainium2 Bass kernel for CustomizeLSTMCell (fused 4-matmul LSTM-like cell).

Math (per token row x of N=100000, H=150):
    pre    = s_in @ W_in + s_out @ W_out + h_in @ U_in + h_out @ U_out
    gate   = sigmoid(pre)
    cell   = gate * last_c + gate * gate = gate * (last_c + gate)
    hidden = gate * tanh(cell)
returns (hidden, cell)

Strategy: data-parallel over tokens across 8 cores (12500 rows/core, padded
to 12544 = 14 * 896). Everything runs feature-major (transposed) on chip:
host packs the four activation tensors as XT[600, 12544] and last_c as
cT[150, 12544] per core; weights concatenate to Wcat[600, 150] and stay
SBUF-resident as the stationary matmul operand. Per 448-token tile the PE
computes preT[150, 448] = Wcat.T @ XT-slice as 2 M-halves (128+22 rows) x 5
K-chunks of 120, with fp16 operands (1 cycle/row, half the X read traffic, ~2^-11 rounding).
ACT does sigmoid/tanh, DVE the elementwise adds/muls; outputs store back
feature-major and the host transposes them back.
"""

import numpy as np

N_TOKENS = 100000
UNITS = 150
N_CORES = 8
ROWS_PER_CORE = N_TOKENS // N_CORES  # 12500
TOK = 448                            # tokens per matmul free dim (>=256)
TOKS_PER_MACRO = 2
MACRO = TOK * TOKS_PER_MACRO         # 896
ROWS_PAD = 12544                     # 14 * 896
N_MACROS = ROWS_PAD // MACRO         # 14
KDIM = 4 * UNITS                     # 600
KCHUNK = 120
N_KCHUNKS = KDIM // KCHUNK           # 5
M0 = 128                             # first output-feature half
M1 = UNITS - M0                      # 22

_CACHE = {}
REPS = 1  # timing aid: repeat the whole macro loop (outputs are idempotent)


def _build_bass():
    import concourse.bacc as bacc
    import concourse.mybir as mybir
    import concourse.tile as tile

    fp32 = mybir.dt.float32
    mmdt = mybir.dt.float16
    nc = bacc.Bacc("TRN2", target_bir_lowering=False, debug=False,
                   num_devices=N_CORES)

    xT = nc.dram_tensor("xT", [KDIM, ROWS_PAD], mmdt, kind="ExternalInput").ap()
    cT = nc.dram_tensor("cT", [UNITS, ROWS_PAD], mmdt, kind="ExternalInput").ap()
    w = nc.dram_tensor("w", [KDIM, UNITS], mmdt, kind="ExternalInput").ap()
    hT_out = nc.dram_tensor("hT_out", [UNITS, ROWS_PAD], mmdt,
                            kind="ExternalOutput").ap()
    cT_out = nc.dram_tensor("cT_out", [UNITS, ROWS_PAD], mmdt,
                            kind="ExternalOutput").ap()

    AF = mybir.ActivationFunctionType

    # [600, T] viewed as [120, 5, T]
    xT_r = xT.rearrange("(k p) t -> p k t", p=KCHUNK)
    w_r = w.rearrange("(k p) d -> p k d", p=KCHUNK)

    with tile.TileContext(nc) as tc:
        with (
            tc.tile_pool(name="wpool", bufs=1) as wpool,
            tc.tile_pool(name="xpool", bufs=3) as xpool,
            tc.tile_pool(name="cpool", bufs=3) as cpool,
            tc.tile_pool(name="opool", bufs=3) as opool,
            tc.tile_pool(name="small", bufs=3) as small,
            tc.tile_pool(name="psum", bufs=3, space="PSUM") as psum_pool,
        ):
            w_tile = wpool.tile([KCHUNK, N_KCHUNKS, UNITS], mmdt)
            nc.sync.dma_start(w_tile[:, :, :], w_r[:, :, :])

            for m in [mm for _ in range(REPS) for mm in range(N_MACROS)]:
                lo, hi = m * MACRO, (m + 1) * MACRO
                x_tile = xpool.tile([KCHUNK, N_KCHUNKS, MACRO], mmdt)
                nc.sync.dma_start(x_tile[:, :, :], xT_r[:, :, lo:hi])
                c0 = cpool.tile([M0, MACRO], mmdt)
                nc.sync.dma_start(c0[:, :], cT[0:M0, lo:hi])
                c1 = cpool.tile([M1, MACRO], mmdt, tag="c1")
                nc.sync.dma_start(c1[:, :], cT[M0:UNITS, lo:hi])

                h0 = opool.tile([M0, MACRO], mmdt, tag="h0")
                h1 = opool.tile([M1, MACRO], mmdt, tag="h1")
                cell0 = opool.tile([M0, MACRO], mmdt, tag="cell0")
                cell1 = opool.tile([M1, MACRO], mmdt, tag="cell1")
                gate0 = small.tile([M0, MACRO], mmdt, tag="gate0")
                gate1 = small.tile([M1, MACRO], mmdt, tag="gate1")

                for t in range(TOKS_PER_MACRO):
                    ts = slice(t * TOK, (t + 1) * TOK)
                    for (mi, mp, msl, gatet, ct, ht, cellt) in (
                        (0, M0, slice(0, M0), gate0, c0, h0, cell0),
                        (1, M1, slice(M0, UNITS), gate1, c1, h1, cell1),
                    ):
                        pre = psum_pool.tile([mp, TOK], fp32, tag=f"pre{mi}")
                        for k in range(N_KCHUNKS):
                            nc.tensor.matmul(
                                pre[:, :],
                                lhsT=w_tile[:, k, msl],
                                rhs=x_tile[:, k, ts],
                                start=(k == 0),
                                stop=(k == N_KCHUNKS - 1),
                            )
                        nc.scalar.activation(gatet[:, ts], pre[:, :], AF.Sigmoid)
                        nc.vector.tensor_add(cellt[:, ts], ct[:, ts], gatet[:, ts])
                        nc.vector.tensor_mul(cellt[:, ts], gatet[:, ts], cellt[:, ts])
                        nc.scalar.activation(ht[:, ts], cellt[:, ts], AF.Tanh)
                        nc.vector.tensor_mul(ht[:, ts], gatet[:, ts], ht[:, ts])

                # Outputs ride the ACT HWDGE ring: HWDGE is FIFO per issuing
                # engine, so putting stores on SP would head-of-line block the
                # next macro's input loads behind this macro's compute.
                if m < N_MACROS - 1:
                    nc.gpsimd.dma_start(hT_out[0:M0, lo:hi], h0[:, :])
                    nc.gpsimd.dma_start(hT_out[M0:UNITS, lo:hi], h1[:, :])
                    nc.gpsimd.dma_start(cT_out[0:M0, lo:hi], cell0[:, :])
                    nc.gpsimd.dma_start(cT_out[M0:UNITS, lo:hi], cell1[:, :])
                else:
                    for t in range(TOKS_PER_MACRO):
                        tl, th_ = lo + t * TOK, lo + (t + 1) * TOK
                        tsl = slice(t * TOK, (t + 1) * TOK)
                        nc.gpsimd.dma_start(hT_out[0:M0, tl:th_], h0[:, tsl])
                        nc.gpsimd.dma_start(hT_out[M0:UNITS, tl:th_], h1[:, tsl])
                        nc.gpsimd.dma_start(cT_out[0:M0, tl:th_], cell0[:, tsl])
                        nc.gpsimd.dma_start(cT_out[M0:UNITS, tl:th_], cell1[:, tsl])

    nc.compile()
    return nc


def _get_nc():
    if "nc" not in _CACHE:
        _CACHE["nc"] = _build_bass()
    return _CACHE["nc"]


def kernel(s_in, s_out, h_in, h_out, last_c,
           w_in_input, w_out_input, u_in_input, u_out_input):
    from concourse.bass_utils import run_bass_kernel_spmd

    nc = _get_nc()

    bf16 = np.float16

    wcat = np.ascontiguousarray(
        np.concatenate([w_in_input, w_out_input, u_in_input, u_out_input],
                       axis=0).astype(np.float32)).astype(bf16)

    in_maps = []
    for c in range(N_CORES):
        rows = slice(c * ROWS_PER_CORE, (c + 1) * ROWS_PER_CORE)
        xT = np.zeros((KDIM, ROWS_PAD), dtype=bf16)
        for j, X in enumerate((s_in, s_out, h_in, h_out)):
            xT[j * UNITS:(j + 1) * UNITS, :ROWS_PER_CORE] = \
                np.asarray(X[rows]).T.astype(bf16)
        cTp = np.zeros((UNITS, ROWS_PAD), dtype=np.float16)
        cTp[:, :ROWS_PER_CORE] = np.asarray(last_c[rows]).T.astype(np.float16)
        in_maps.append({"xT": xT, "cT": cTp, "w": wcat})

    res = run_bass_kernel_spmd(nc, in_maps, core_ids=list(range(N_CORES)))

    hidden = np.concatenate(
        [res.results[c]["hT_out"][:, :ROWS_PER_CORE].T for c in range(N_CORES)],
        axis=0).astype(np.float32)
    cell = np.concatenate(
        [res.results[c]["cT_out"][:, :ROWS_PER_CORE].T for c in range(N_CORES)],
        axis=0).astype(np.float32)
    return np.ascontiguousarray(hidden), np.ascontiguousarray(cell)



# revision 3
# speedup vs baseline: 1.1346x; 1.1346x over previous
"""Trainium2 Bass kernel for CustomizeLSTMCell (fused 4-matmul LSTM-like cell).

Math (per token row x of N=100000, H=150):
    pre    = s_in @ W_in + s_out @ W_out + h_in @ U_in + h_out @ U_out
    gate   = sigmoid(pre)
    cell   = gate * last_c + gate * gate = gate * (last_c + gate)
    hidden = gate * tanh(cell)
returns (hidden, cell)

Strategy: data-parallel over tokens across 8 cores (12500 rows/core, padded to
12544 = 98 blocks of 128). Token-major on chip: each 128-token block maps
tokens to partitions, features to the free dim, so every elementwise/activation
op runs on all 128 lanes. The matmul makes the streaming X the *stationary*
operand (lhsT = xT chunk [120, 128]) and the SBUF-resident weights the moving
one (rhs = W chunk [120, 150]), so each Matmult only streams 150 columns.

Layouts (per core):
  xT    [600, 12544]  fp16, feature-major (host transposes)    -> matmul lhsT
  cP    [128, 98*150] fp16, block-interleaved token-major      -> elementwise
  w     [120, 750]    fp16, w[p, k*150+d] = Wcat[k*120+p, d]   -> matmul rhs
  hP/cellP like cP    (host un-interleaves)
All DMAs have >=1500B contiguous runs (>=512B avoids the 2x DMA penalty).
Loads ride the SP HWDGE ring; stores ride Pool SWDGE from per-chunk dedicated
buffers so nothing ever waits on buffer reuse and DMA stays saturated.
"""

import numpy as np

N_TOKENS = 100000
UNITS = 150
N_CORES = 8
ROWS_PER_CORE = N_TOKENS // N_CORES  # 12500
BLK = 128                            # tokens per block (partition dim)
N_BLOCKS = 98
ROWS_PAD = BLK * N_BLOCKS            # 12544
KDIM = 4 * UNITS                     # 600
KCHUNK = 120
N_KCHUNKS = KDIM // KCHUNK           # 5
GROUP_BLOCKS = 3                     # blocks per PSUM group (450 fp32 cols)
CHUNK_BLOCKS = 12                    # blocks per DMA chunk (4 groups)
# 8 full chunks of 12 blocks + 1 tail chunk of 2 blocks
CHUNKS = [(i * CHUNK_BLOCKS, CHUNK_BLOCKS) for i in range(8)] + [(96, 2)]
FREE_W = N_BLOCKS * UNITS            # 14700

_CACHE = {}


def _build_bass():
    import concourse.bacc as bacc
    import concourse.mybir as mybir
    import concourse.tile as tile

    fp32 = mybir.dt.float32
    fp16 = mybir.dt.float16
    nc = bacc.Bacc("TRN2", target_bir_lowering=False, debug=False,
                   num_devices=N_CORES)

    xT = nc.dram_tensor("xT", [KDIM, ROWS_PAD], fp16, kind="ExternalInput").ap()
    cP = nc.dram_tensor("cP", [BLK, FREE_W], fp16, kind="ExternalInput").ap()
    w = nc.dram_tensor("w", [KCHUNK, N_KCHUNKS * UNITS], fp16,
                       kind="ExternalInput").ap()
    hP = nc.dram_tensor("hP", [BLK, FREE_W], fp16, kind="ExternalOutput").ap()
    cellP = nc.dram_tensor("cellP", [BLK, FREE_W], fp16,
                           kind="ExternalOutput").ap()

    AF = mybir.ActivationFunctionType

    # [600, T] viewed as [120, 5, T]
    xT_r = xT.rearrange("(k p) t -> p k t", p=KCHUNK)

    with tile.TileContext(nc) as tc:
        with (
            tc.tile_pool(name="wpool", bufs=1) as wpool,
            tc.tile_pool(name="xpool", bufs=3) as xpool,
            tc.tile_pool(name="cpool", bufs=3) as cpool,
            tc.tile_pool(name="gpool", bufs=2) as gpool,
            tc.tile_pool(name="hpool", bufs=len(CHUNKS)) as hpool,
            tc.tile_pool(name="cellpool", bufs=len(CHUNKS)) as cellpool,
            tc.tile_pool(name="psum", bufs=6, space="PSUM") as psum_pool,
        ):
            w_tile = wpool.tile([KCHUNK, N_KCHUNKS * UNITS], fp16)
            nc.sync.dma_start(w_tile[:, :], w[:, :])

            for sb, nb in CHUNKS:
                tg = "" if nb == CHUNK_BLOCKS else "t"
                xw, cw = nb * BLK, nb * UNITS
                x0, c0 = sb * BLK, sb * UNITS

                x_tile = xpool.tile([KCHUNK, N_KCHUNKS, xw], fp16, tag="x" + tg)
                nc.sync.dma_start(x_tile[:, :, :], xT_r[:, :, x0:x0 + xw])
                c_tile = cpool.tile([BLK, cw], fp16, tag="c" + tg)
                nc.sync.dma_start(c_tile[:, :], cP[:, c0:c0 + cw])

                gate = gpool.tile([BLK, cw], fp16, tag="g" + tg)
                h_t = hpool.tile([BLK, cw], fp16, tag="h" + tg)
                cell_t = cellpool.tile([BLK, cw], fp16, tag="cc" + tg)

                for g0 in range(0, nb, GROUP_BLOCKS):
                    gb = min(GROUP_BLOCKS, nb - g0)
                    pre_full = psum_pool.tile([BLK, GROUP_BLOCKS * UNITS],
                                              fp32, tag="pre")
                    pre = pre_full[:, :gb * UNITS]
                    for b in range(gb):
                        blk = g0 + b
                        for k in range(N_KCHUNKS):
                            nc.tensor.matmul(
                                pre[:, b * UNITS:(b + 1) * UNITS],
                                lhsT=x_tile[:, k, blk * BLK:(blk + 1) * BLK],
                                rhs=w_tile[:, k * UNITS:(k + 1) * UNITS],
                                start=(k == 0),
                                stop=(k == N_KCHUNKS - 1),
                            )
                    csl = slice(g0 * UNITS, (g0 + gb) * UNITS)
                    nc.scalar.activation(gate[:, csl], pre[:, :], AF.Sigmoid)
                    # tmp = last_c + gate (h_t doubles as scratch)
                    nc.vector.tensor_add(h_t[:, csl], c_tile[:, csl],
                                         gate[:, csl])
                    nc.vector.tensor_mul(cell_t[:, csl], gate[:, csl],
                                         h_t[:, csl])

                # tanh + final mul over the whole chunk (fewer, wider ops)
                nc.scalar.activation(h_t[:, :], cell_t[:, :], AF.Tanh)
                nc.vector.tensor_mul(h_t[:, :], gate[:, :], h_t[:, :])

                # Stores ride Pool SWDGE: own engine, no HWDGE head-of-line
                # blocking of the input loads on SP.
                nc.gpsimd.dma_start(hP[:, c0:c0 + cw], h_t[:, :])
                nc.gpsimd.dma_start(cellP[:, c0:c0 + cw], cell_t[:, :])

    nc.compile()
    return nc


def _get_nc():
    if "nc" not in _CACHE:
        _CACHE["nc"] = _build_bass()
    return _CACHE["nc"]


def kernel(s_in, s_out, h_in, h_out, last_c,
           w_in_input, w_out_input, u_in_input, u_out_input):
    from concourse.bass_utils import run_bass_kernel_spmd

    nc = _get_nc()

    f16 = np.float16

    wcat = np.concatenate(
        [w_in_input, w_out_input, u_in_input, u_out_input],
        axis=0).astype(np.float32)
    # w[p, k*150+d] = Wcat[k*120+p, d]
    wp = np.ascontiguousarray(
        wcat.reshape(N_KCHUNKS, KCHUNK, UNITS).transpose(1, 0, 2)
        .reshape(KCHUNK, N_KCHUNKS * UNITS)).astype(f16)

    in_maps = []
    for c in range(N_CORES):
        rows = slice(c * ROWS_PER_CORE, (c + 1) * ROWS_PER_CORE)
        xT = np.zeros((KDIM, ROWS_PAD), dtype=f16)
        for j, X in enumerate((s_in, s_out, h_in, h_out)):
            xT[j * UNITS:(j + 1) * UNITS, :ROWS_PER_CORE] = \
                np.asarray(X[rows]).T.astype(f16)
        # block-interleaved token-major: cP[p, b*150+d] = c[b*128+p, d]
        cpad = np.zeros((ROWS_PAD, UNITS), dtype=f16)
        cpad[:ROWS_PER_CORE] = np.asarray(last_c[rows]).astype(f16)
        cPm = np.ascontiguousarray(
            cpad.reshape(N_BLOCKS, BLK, UNITS).transpose(1, 0, 2)
            .reshape(BLK, FREE_W))
        in_maps.append({"xT": xT, "cP": cPm, "w": wp})

    res = run_bass_kernel_spmd(nc, in_maps, core_ids=list(range(N_CORES)))

    def unpack(name):
        outs = []
        for c in range(N_CORES):
            m = np.asarray(res.results[c][name])
            outs.append(m.reshape(BLK, N_BLOCKS, UNITS).transpose(1, 0, 2)
                        .reshape(ROWS_PAD, UNITS)[:ROWS_PER_CORE])
        return np.ascontiguousarray(
            np.concatenate(outs, axis=0).astype(np.float32))

    return unpack("hP"), unpack("cellP")
